# revision 2
# baseline (speedup 1.0000x reference)
"""Trainium2 Bass kernel for nn_KnowledgeEncoding_51153060496172.

Mathematical note: in the reference, both branch outputs are a softmax-
weighted sum over an axis on which the value tensor is constant:

  updated_v_nodes[b,r,i,:] = sum_j softmax_j(...)[b,r,i,j] * img[b,i,:]
                           = img[b,i,:]            (softmax sums to 1 over j)
  updated_t_nodes[b,r,t,:] = sum_{r'} softmax_{r'}(...) * hist_embed[b,r,t,:]
                           = hist_embed[b,r,t,:]

so the exact output is a broadcast of `img` over the round axis and a copy
of `hist_embed`. The kernel therefore reduces to pure DMA data movement,
sharded over the 40 (b, r) pairs across 8 cores (5 rounds per core; the
two cores serving the same batch element each receive that element's img
slice).
"""

import os
import sys

import numpy as np

sys.path.insert(0, "/opt/trn_rl_repo")

import concourse.bass as bass  # noqa: E402
import concourse.mybir as mybir  # noqa: E402
from concourse.bass_utils import run_bass_kernel_spmd  # noqa: E402

B, R, N, T, H, F = 4, 10, 36, 10, 512, 2048
NCORES = 8
RPC = (B * R) // NCORES  # rounds per core = 5

_FP32 = mybir.dt.float32

VARIANT = os.environ.get("KE_VARIANT", "dram6")


def _build_dram6(nc, img_s, hist_s, out_v, out_t):
    """Direct DRAM->DRAM copies: 5x img replica + 1x hist, all on SP HWDGE."""
    with nc.Block() as block, nc.semaphore("dma_sem") as dma_sem:

        @block.sync
        def _(sync):
            for r in range(RPC):
                sync.dma_start(out=out_v[r], in_=img_s[:]).then_inc(dma_sem, 16)
            sync.dma_start(out=out_t[:], in_=hist_s[:]).then_inc(dma_sem, 16)
            sync.wait_ge(dma_sem, 16 * (RPC + 1))


def _build_dram_2eng(nc, img_s, hist_s, out_v, out_t):
    """DRAM->DRAM split over the two HWDGE rings (SP + ACT)."""
    with nc.Block() as block, nc.semaphore("dma_sem") as dma_sem:

        @block.sync
        def _(sync):
            for r in range(0, RPC, 2):
                sync.dma_start(out=out_v[r], in_=img_s[:]).then_inc(dma_sem, 16)
            sync.dma_start(out=out_t[:], in_=hist_s[:]).then_inc(dma_sem, 16)
            sync.wait_ge(dma_sem, 16 * (RPC + 1))

        @block.scalar
        def _(scalar):
            for r in range(1, RPC, 2):
                scalar.dma_start(out=out_v[r], in_=img_s[:]).then_inc(dma_sem, 16)


def _build_bounce(nc, img_s, hist_s, out_v, out_t):
    """Load img+hist to SBUF once, then fan out stores (halves HBM reads)."""
    IMG_P, IMG_W = 128, (N * F) // 128  # 36*2048 = 73728 = 128*576
    HST_P, HST_W = 128, (RPC * T * H) // 128  # 25600 = 128*200
    img2 = img_s.rearrange("(p w) -> p w", p=IMG_P)
    hist2 = hist_s.rearrange("(p w) -> p w", p=HST_P)
    with (
        nc.sbuf_tensor([IMG_P, IMG_W], _FP32) as img_t,
        nc.sbuf_tensor([HST_P, HST_W], _FP32) as hist_t,
        nc.Block() as block,
        nc.semaphore("dma_sem") as dma_sem,
    ):

        @block.sync
        def _(sync):
            sync.dma_start(out=img_t[:], in_=img2[:]).then_inc(dma_sem, 16)
            sync.dma_start(out=hist_t[:], in_=hist2[:]).then_inc(dma_sem, 16)
            sync.wait_ge(dma_sem, 32)
            for r in range(RPC):
                sync.dma_start(
                    out=out_v[r].rearrange("(p w) -> p w", p=IMG_P), in_=img_t[:]
                ).then_inc(dma_sem, 16)
            sync.dma_start(
                out=out_t[:].rearrange("(p w) -> p w", p=HST_P), in_=hist_t[:]
            ).then_inc(dma_sem, 16)
            sync.wait_ge(dma_sem, 32 + 16 * (RPC + 1))


def _build_bounce_2eng(nc, img_s, hist_s, out_v, out_t):
    """SBUF bounce with stores split across SP + ACT HWDGE rings."""
    IMG_P, IMG_W = 128, (N * F) // 128
    HST_P, HST_W = 128, (RPC * T * H) // 128
    img2 = img_s.rearrange("(p w) -> p w", p=IMG_P)
    hist2 = hist_s.rearrange("(p w) -> p w", p=HST_P)
    with (
        nc.sbuf_tensor([IMG_P, IMG_W], _FP32) as img_t,
        nc.sbuf_tensor([HST_P, HST_W], _FP32) as hist_t,
        nc.Block() as block,
        nc.semaphore("ld_sem") as ld_sem,
        nc.semaphore("st_sem") as st_sem,
    ):

        @block.sync
        def _(sync):
            sync.dma_start(out=img_t[:], in_=img2[:]).then_inc(ld_sem, 16)
            sync.dma_start(out=hist_t[:], in_=hist2[:]).then_inc(ld_sem, 16)
            sync.wait_ge(ld_sem, 32)
            for r in range(0, RPC, 2):
                sync.dma_start(
                    out=out_v[r].rearrange("(p w) -> p w", p=IMG_P), in_=img_t[:]
                ).then_inc(st_sem, 16)
            sync.dma_start(
                out=out_t[:].rearrange("(p w) -> p w", p=HST_P), in_=hist_t[:]
            ).then_inc(st_sem, 16)
            sync.wait_ge(st_sem, 16 * (RPC + 1))

        @block.scalar
        def _(scalar):
            scalar.wait_ge(ld_sem, 16)
            for r in range(1, RPC, 2):
                scalar.dma_start(
                    out=out_v[r].rearrange("(p w) -> p w", p=IMG_P), in_=img_t[:]
                ).then_inc(st_sem, 16)


_BUILDERS = {
    "dram6": _build_dram6,
    "dram_2eng": _build_dram_2eng,
    "bounce": _build_bounce,
    "bounce_2eng": _build_bounce_2eng,
}


def _build_nc():
    nc = bass.Bass("TRN2")
    img_s = nc.declare_dram_parameter("img_s", [N * F], _FP32, isOutput=False)
    hist_s = nc.declare_dram_parameter("hist_s", [RPC * T * H], _FP32, isOutput=False)
    out_v = nc.declare_dram_parameter("out_v", [RPC, N * F], _FP32, isOutput=True)
    out_t = nc.declare_dram_parameter("out_t", [RPC * T * H], _FP32, isOutput=True)
    _BUILDERS[VARIANT](nc, img_s, hist_s, out_v, out_t)
    return nc


_NC = None


def _get_nc():
    global _NC
    if _NC is None:
        _NC = _build_nc()
    return _NC


def kernel(**inputs) -> tuple[np.ndarray, np.ndarray]:
    img = np.ascontiguousarray(np.asarray(inputs["img"], dtype=np.float32))
    hist = np.ascontiguousarray(np.asarray(inputs["hist_embed"], dtype=np.float32))

    nc = _get_nc()

    in_maps = []
    for c in range(NCORES):
        b, half = divmod(c, 2)
        r0 = half * RPC
        in_maps.append(
            {
                "img_s": img[b].reshape(N * F),
                "hist_s": hist[b, r0 : r0 + RPC].reshape(RPC * T * H),
            }
        )

    res = run_bass_kernel_spmd(
        nc,
        in_maps,
        core_ids=list(range(NCORES)),
        trace=bool(int(os.environ.get("KE_TRACE", "0"))),
    )

    out_v = np.empty((B, R, N, F), np.float32)
    out_t = np.empty((B, R, T, H), np.float32)
    for c in range(NCORES):
        b, half = divmod(c, 2)
        r0 = half * RPC
        out_v[b, r0 : r0 + RPC] = res.results[c]["out_v"].reshape(RPC, N, F)
        out_t[b, r0 : r0 + RPC] = res.results[c]["out_t"].reshape(RPC, T, H)

    kernel.last_exec_time_ns = res.exec_time_ns
    return out_v, out_t


kernel.last_exec_time_ns = None
